# revision 10
# baseline (speedup 1.0000x reference)
"""Trainium2 Bass kernel v8 — paired scores, split h2 PV, lean phase A.

Multi-head attention (B=2, N=4096, D=768, H=12, d_head=64) on 8 NeuronCores.
Data-parallel over batch (4 cores per element), tensor-parallel over heads
(3 heads per core). Host sums the 4 partial outputs per batch element and
adds the bias.

v8 changes over v7 (481388 ns) / v6.1 (474353 ns):

1. h2's PV matmuls are row-tiled into two concurrent 64-key halves
   accumulating in SEPARATE PSUM banks (o2a/o2b; concurrent same-bank
   accumulation aborts the device). The h2 loop then contains only
   (64,128)-tile matmuls - no PE array reconfiguration (each half<->full
   transition costs ~95 ns). o2a+o2b are combined on the way out of PSUM.

2. Phase A projects q2/k2 ONCE (combined 128-col stationary [q2|k2]) and
   duplicates the halves into both partition ranges with SBUF->SBUF DMAs,
   dropping one 128-col projection block per segment (~13 us PE).

3. Output-projection token chunks are emitted at the h01->h2 and qb
   boundaries, where the normalize dependency chain otherwise starves
   the PE (the v7 trace showed a HAM re-throttle to 1.2 GHz every qb).

4. fp16 host-packed inputs (x pre-arranged partition-major), merged V
   tile, batched h0+h1 normalize (one reciprocal + one broadcast).

Layouts (per core, fp16):
  kT01/qT01 [128, N]: h0 dims rows 0:64, h1 rows 64:128 (K pre-scaled by
      alpha*SCALE host-side).
  kT2/qT2   [128, N]: h2 duplicated into both partition halves (via DMA).
  v16 [128, NKC, 3, 128]: keys on partitions; block 0 = h0 (V cols 0:64,
      ones col 64), block 1 = h2 (same), block 2 = h1 (ones col 0, V cols
      64:128 so o1 rows land partition-aligned with A01's h1 half).
  P [128, 2, 2, QB] per chunk-pair: [keys, parity, head, q].
  A01 [128, N] (h0 rows 0:64, h1 rows 64:128), A2 [64, N].
wqkv host layout [128, 6, 576]: [q01 | k01*KS | q2 | k2*KS | vh0 vh2 vh1],
  pre-transposed partition-major.
wout host layout [192, 768]: [W_h0;W_h1;W_h2]. Bias added on host.
x host layout [128, NSEG, 6, SEG] partition-major segments.
"""

import numpy as np

import concourse.bass as bass
import concourse.tile as tile
from concourse import mybir, bacc
from concourse.bass_utils import run_bass_kernel_spmd

F32 = mybir.dt.float32
F16 = mybir.dt.float16
EXP = mybir.ActivationFunctionType.Exp

N_CORES = 8
B = 2
N = 4096
D = 768
H = 12
HD = 64
SCALE = HD ** -0.5
DC = D // 128        # 6 contraction chunks
NKC = N // 128       # 32 key chunks
NCP = NKC // 2       # 16 chunk pairs
QB = 512             # query block
NQB = N // QB        # 8
NSEG = 4
SEG = N // NSEG
WCOLS = 576

# EXP32 fit: g(u) = ((u + EXP_A)^2 + EXP_B)^32 ~= exp(u/ALPHA)
ALPHA = 0.0230805526
EXP_A = 0.6770127392
EXP_B = 0.5415557589
KSCALE = ALPHA * SCALE          # folded into K projection columns host-side
ACT_SCALE = 1.0 / ALPHA         # ScalarE: exp(ACT_SCALE * s)

DVE_SHARE = 0.48                # fraction of exp tiles routed to the DVE

TRACE = False
TRACE_ALL_CORES = False
LAST_RESULT = None

_nc_cache = None
_exp32_op = None


def _register_exp32():
    """Register the EXP32 custom DVE op at runtime (idempotent)."""
    global _exp32_op
    if _exp32_op is not None:
        return _exp32_op
    import concourse.dve_ops as dve_ops
    from concourse.dve_spec import Spec, Src0, C1, C2, sq, lower
    from concourse.dve_uop import DveOpSpec

    name = "EXP32_SQCHAIN"
    for op in dve_ops.OPS:
        if op.name == name:
            _exp32_op = op
            return op
    body = sq(sq(sq(sq(sq(sq(Src0 + C1) + C2)))))
    spec = Spec(
        body=body,
        reference=lambda in0, in1, s0, s1, imm2: (
            ((in0.astype(np.float32) + s1) ** 2 + imm2) ** 32
        ).astype(np.float32),
    )
    row = dve_ops._CUSTOM_DVE_ROW_BASE + len(dve_ops.OPS)
    assert row < 0x20
    shas = {
        ver: DveOpSpec(
            name=name, opcode=row, uops=lower(spec, ver=ver), rd1_en=False
        ).sha(ver)
        for ver in ("v3", "v4")
    }
    op = dve_ops.DveOp(name, spec, subdim=False, uops_sha=shas)
    dve_ops.OPS.append(op)
    dve_ops._SUB_OPCODE_FOR_NAME[name] = row
    dve_ops.CUSTOM_DVE_SPECS[name] = spec
    _exp32_op = op
    return op


def _build_module():
    nc = bacc.Bacc("TRN2", target_bir_lowering=False, debug=False,
                   num_devices=N_CORES)
    x_d = nc.dram_tensor("x", [128, NSEG, DC * SEG], F16, kind="ExternalInput")
    wqkv_d = nc.dram_tensor("wqkv", [128, DC * WCOLS], F16,
                            kind="ExternalInput")
    wout_d = nc.dram_tensor("wout", [192, D], F16, kind="ExternalInput")
    y_d = nc.dram_tensor("y", [N, D], F16, kind="ExternalOutput")

    with tile.TileContext(nc) as tc:
        _emit(nc, tc, x_d, wqkv_d, wout_d, y_d)
    nc.compile()
    return nc


def _emit(nc, tc, x_d, wqkv_d, wout_d, y_d):
    from contextlib import ExitStack
    exp32 = _register_exp32()
    ctx = ExitStack()
    with ctx:
        weights = ctx.enter_context(tc.tile_pool(name="weights", bufs=1))
        qkvp = ctx.enter_context(tc.tile_pool(name="qkv", bufs=1))
        apool = ctx.enter_context(tc.tile_pool(name="attnout", bufs=1))

        wqkv = weights.tile([128, DC, WCOLS], F16, tag="wqkv")
        W01 = weights.tile([128, D], F16, tag="W01")
        W2 = weights.tile([64, D], F16, tag="W2")
        qT01 = qkvp.tile([128, N], F16, tag="qT01")
        kT01 = qkvp.tile([128, N], F16, tag="kT01")
        qT2 = qkvp.tile([128, N], F16, tag="qT2")
        kT2 = qkvp.tile([128, N], F16, tag="kT2")
        # blocks: 0 = h0, 1 = h2, 2 = h1
        v16 = qkvp.tile([128, NKC, 3, 128], F16, tag="v16")
        VH0, VH2, VH1 = 0, 1, 2
        A01 = apool.tile([128, N], F16, tag="A01")
        A2 = apool.tile([64, N], F16, tag="A2")

        nc.gpsimd.memset(v16[:, :, 0:2, 65:128], 0.0)   # h0, h2 blocks
        nc.gpsimd.memset(v16[:, :, 0:2, 64:65], 1.0)
        nc.gpsimd.memset(v16[:, :, 2, 1:64], 0.0)       # h1 block
        nc.gpsimd.memset(v16[:, :, 2, 0:1], 1.0)

        # ================= phase A: DMA + projections ====================
        with tc.tile_pool(name="xT", bufs=2) as xTp, \
             tc.tile_pool(name="vps", bufs=2, space=bass.MemorySpace.PSUM) as vps, \
             tc.tile_pool(name="qkps", bufs=2, space=bass.MemorySpace.PSUM) as qkps:
            nc.sync.dma_start(wqkv[:], wqkv_d.ap())
            nc.sync.dma_start(W01[:], wout_d.ap()[0:128, :])
            nc.sync.dma_start(W2[:], wout_d.ap()[128:192, :])

            SEGC = SEG // 128
            qk_eng = [nc.vector, nc.scalar]
            qk_i = 0
            for seg in range(NSEG):
                t0 = seg * SEGC
                col0 = seg * SEG
                xT = xTp.tile([128, DC, SEG], F16, tag="xT")
                nc.sync.dma_start(xT[:], x_d.ap()[:, seg, :])
                # block cols: q01 0:128 | k01 128:256 | [q2|k2] 256:384 |
                #             v 384:576
                for ci, dst in ((1, kT01), (2, None), (-1, None),
                                (0, qT01)):
                    if ci == -1:
                        for t in range(SEGC):
                            kc = t0 + t
                            acc = vps.tile([128, 192], F32, tag="vps")
                            for c in range(DC):
                                nc.tensor.matmul(acc[:],
                                                 xT[:, c, t * 128:(t + 1) * 128],
                                                 wqkv[:, c, 384:576],
                                                 start=(c == 0), stop=(c == DC - 1))
                            # acc cols: [v_h0 | v_h2 | v_h1]
                            if kc % 2 == 0:
                                nc.scalar.copy(v16[:, kc, 0:2, 0:64],
                                               acc[:, 0:128])
                                nc.vector.tensor_copy(v16[:, kc, 2, 64:128],
                                                      acc[:, 128:192])
                            else:
                                nc.vector.tensor_copy(v16[:, kc, 0:2, 0:64],
                                                      acc[:, 0:128])
                                nc.scalar.copy(v16[:, kc, 2, 64:128],
                                               acc[:, 128:192])
                        continue
                    c0 = 128 * ci
                    for nb in range(SEG // 512):
                        acc = qkps.tile([128, 512], F32, tag="qkps")
                        for c in range(DC):
                            nc.tensor.matmul(acc[:], wqkv[:, c, c0:c0 + 128],
                                             xT[:, c, nb * 512:(nb + 1) * 512],
                                             start=(c == 0), stop=(c == DC - 1))
                        cc = col0 + nb * 512
                        if ci == 2:
                            # combined [q2|k2] projection: q2 dims on
                            # partitions 0:64, k2 dims on 64:128
                            eng = qk_eng[qk_i % 2]
                            qk_i += 1
                            if eng is nc.scalar:
                                nc.scalar.copy(qT2[0:64, cc:cc + 512],
                                               acc[0:64, :])
                                nc.vector.tensor_copy(kT2[64:128, cc:cc + 512],
                                                      acc[64:128, :])
                            else:
                                nc.vector.tensor_copy(qT2[0:64, cc:cc + 512],
                                                      acc[0:64, :])
                                nc.scalar.copy(kT2[64:128, cc:cc + 512],
                                               acc[64:128, :])
                            continue
                        eng = qk_eng[qk_i % 2]
                        qk_i += 1
                        if eng is nc.scalar:
                            nc.scalar.copy(dst[:, cc:cc + 512], acc[:])
                        else:
                            eng.tensor_copy(dst[:, cc:cc + 512], acc[:])
                # duplicate h2's q/k into the other partition half
                nc.sync.dma_start(qT2[64:128, col0:col0 + SEG],
                                  qT2[0:64, col0:col0 + SEG])
                nc.sync.dma_start(kT2[0:64, col0:col0 + SEG],
                                  kT2[64:128, col0:col0 + SEG])

        # ========= phase B: attention + fused output projection ==========
        # PSUM budget (8 banks): tag "s" ring 3 x [128,2,QB] f32 (6 banks,
        # also hosts the [128,D] y-projection tiles) + tag "o" ring 2 x
        # [128,QB] f32 (2 banks; o0/o1 in the h01 loop, o2a/o2b in h2's).
        with tc.tile_pool(name="sps", bufs=3, space=bass.MemorySpace.PSUM) as sps, \
             tc.tile_pool(name="ops", bufs=2, space=bass.MemorySpace.PSUM) as ops, \
             tc.tile_pool(name="pp", bufs=5) as pp, \
             tc.tile_pool(name="p2p", bufs=5) as p2p, \
             tc.tile_pool(name="ysbp", bufs=3) as ysbp, \
             tc.tile_pool(name="rp", bufs=4) as rp, \
             tc.tile_pool(name="o2sp", bufs=2) as o2sp, \
             tc.tile_pool(name="rbp", bufs=2) as rbp:
            pending = []
            exp_ctr = [0]

            def emit_exp(p_ap, s_ap, force_act=False):
                if force_act:
                    nc.scalar.activation(p_ap, s_ap, EXP, scale=ACT_SCALE)
                    return
                i = exp_ctr[0]
                exp_ctr[0] += 1
                if int((i + 1) * DVE_SHARE) - int(i * DVE_SHARE) == 1:
                    nc.vector._custom_dve(exp32, out=p_ap, in0=s_ap,
                                          s1=EXP_A, imm2=EXP_B)
                else:
                    nc.scalar.activation(p_ap, s_ap, EXP, scale=ACT_SCALE)

            y_eng = [0]

            def emit_y(t):
                ts = slice(t * 128, (t + 1) * 128)
                y = sps.tile([128, D], F32, tag="s", name="y")
                for c0, c1 in ((0, 512), (512, 768)):
                    nc.tensor.matmul(y[:, c0:c1], A01[:, ts], W01[:, c0:c1],
                                     start=True, stop=False)
                    nc.tensor.matmul(y[:, c0:c1], A2[:, ts], W2[:, c0:c1],
                                     start=False, stop=True)
                ysb = ysbp.tile([128, D], F16, tag="ysb", name="ysb")
                if y_eng[0] % 2 == 0:
                    nc.scalar.copy(ysb[:], y[:])
                else:
                    nc.vector.tensor_copy(ysb[:], y[:])
                y_eng[0] += 1
                nc.sync.dma_start(y_d.ap()[ts, :], ysb[:])

            def normalize01(o0, o1, qs):
                den = rp.tile([1, 2, QB], F32, tag="den", name="den")
                nc.vector.tensor_copy(den[:, 0, :], o0[64:65, :])
                nc.vector.tensor_copy(den[:, 1, :], o1[0:1, :])
                rc = rp.tile([1, 2, QB], F32, tag="rc", name="rc")
                nc.vector.reciprocal_approx_fast(rc[:], den[:])
                rcb = rbp.tile([128, 2, QB], F32, tag="rcb", name="rcb")
                nc.gpsimd.partition_broadcast(rcb[:], rc[:])
                nc.vector.tensor_mul(A01[0:64, qs], o0[0:64, :],
                                     rcb[0:64, 0, :])
                nc.vector.tensor_mul(A01[64:128, qs], o1[64:128, :],
                                     rcb[64:128, 1, :])

            def normalize2(o2a, o2b, qs):
                # combine the two half-PV accumulators on the way out
                t1 = o2sp.tile([65, QB], F32, tag="t1", name="t1")
                nc.scalar.copy(t1[:], o2a[0:65, :])
                t2 = o2sp.tile([65, QB], F32, tag="t2", name="t2")
                nc.vector.tensor_add(t2[:], t1[:], o2b[0:65, :])
                # the custom-DVE recip mishandles nonzero base partitions:
                # copy the denominator row down to partition 0 first.
                den2 = rp.tile([1, QB], F32, tag="den2", name="den2")
                nc.vector.tensor_copy(den2[:], t2[64:65, :])
                rc = rp.tile([1, QB], F32, tag="rc2", name="rc2")
                nc.vector.reciprocal_approx_fast(rc[:], den2[:])
                rcb = rbp.tile([128, QB], F32, tag="rcb2", name="rcb2")
                nc.gpsimd.partition_broadcast(rcb[:], rc[:])
                nc.vector.tensor_mul(A2[0:64, qs], t2[0:64, :], rcb[0:64, :])

            def pv01(o0, o1, P, cp):
                for par in (0, 1):
                    st = (cp == 0 and par == 0)
                    sp = (cp == NCP - 1 and par == 1)
                    nc.tensor.matmul(o0[:], v16[:, 2 * cp + par, VH0, :],
                                     P[:, par, 0, :], start=st, stop=sp)
                    nc.tensor.matmul(o1[:], v16[:, 2 * cp + par, VH1, :],
                                     P[:, par, 1, :], start=st, stop=sp)

            def pv2(o2a, o2b, p2, cp):
                for par in (0, 1):
                    kc = 2 * cp + par
                    st = (cp == 0 and par == 0)
                    sp = (cp == NCP - 1 and par == 1)
                    nc.tensor.matmul(o2a[:], v16[0:64, kc, VH2, :],
                                     p2[0:64, par, :], start=st, stop=sp)
                    nc.tensor.matmul(o2b[:], v16[64:128, kc, VH2, :],
                                     p2[64:128, par, :], start=st, stop=sp)

            for qb in range(NQB):
                q0 = qb * QB
                qs = slice(q0, q0 + QB)
                # ---- heads 0+1 ------------------------------------------
                o0 = ops.tile([128, QB], F32, tag="o", name="o0")
                o1 = ops.tile([128, QB], F32, tag="o", name="o1")
                lag01 = []
                for cp in range(NCP):
                    if cp % 2 == 0:
                        while len(lag01) > 2:
                            pv01(o0, o1, *lag01.pop(0))
                    P = pp.tile([128, 2, 2, QB], F16, tag="p", name="P")
                    tail = cp >= NCP - 2
                    for par in (0, 1):
                        kc = 2 * cp + par
                        ks = slice(kc * 128, (kc + 1) * 128)
                        s = sps.tile([128, 2, QB], F32, tag="s", name="s")
                        nc.tensor.matmul(s[:, 0, :], kT01[0:64, ks],
                                         qT01[0:64, qs], start=True, stop=True)
                        nc.tensor.matmul(s[:, 1, :], kT01[64:128, ks],
                                         qT01[64:128, qs], start=True, stop=True)
                        emit_exp(P[:, par, :, :], s[:], force_act=tail)
                    lag01.append((P, cp))
                for ent in lag01:
                    pv01(o0, o1, *ent)
                normalize01(o0, o1, qs)
                # boundary filler: output projection for older token chunks
                if pending:
                    emit_y(pending.pop(0))
                if pending:
                    emit_y(pending.pop(0))
                # ---- head 2 (all (64,128)-tile ops) ----------------------
                o2a = ops.tile([128, QB], F32, tag="o", name="o2a")
                o2b = ops.tile([128, QB], F32, tag="o", name="o2b")
                lag2 = []
                for cp in range(NCP):
                    if cp % 2 == 0:
                        while len(lag2) > 2:
                            pv2(o2a, o2b, *lag2.pop(0))
                    s2 = sps.tile([128, 2, QB], F32, tag="s", name="s2")
                    ka = slice((2 * cp) * 128, (2 * cp + 1) * 128)
                    kb = slice((2 * cp + 1) * 128, (2 * cp + 2) * 128)
                    nc.tensor.matmul(s2[:, 0, :], kT2[0:64, ka],
                                     qT2[0:64, qs], start=True, stop=True)
                    nc.tensor.matmul(s2[:, 1, :], kT2[64:128, kb],
                                     qT2[64:128, qs], start=True, stop=True)
                    p2 = p2p.tile([128, 2, QB], F16, tag="p2", name="p2")
                    emit_exp(p2[:], s2[:], force_act=(cp >= NCP - 2))
                    lag2.append((p2, cp))
                    if pending and cp == 7:
                        emit_y(pending.pop(0))
                for ent in lag2:
                    pv2(o2a, o2b, *ent)
                normalize2(o2a, o2b, qs)
                if pending:
                    emit_y(pending.pop(0))
                pending.extend(range(qb * (QB // 128), (qb + 1) * (QB // 128)))
            for t in pending:
                emit_y(t)


def _get_nc():
    global _nc_cache
    if _nc_cache is None:
        _nc_cache = _build_module()
    return _nc_cache


def kernel(x, W_qkv, W_out, b_out):
    global LAST_RESULT
    x = np.asarray(x, dtype=np.float32)
    W_qkv = np.asarray(W_qkv, dtype=np.float32)
    W_out = np.asarray(W_out, dtype=np.float32)
    b_out = np.asarray(b_out, dtype=np.float32)

    in_maps = []
    for c in range(N_CORES):
        b, j = divmod(c, 4)
        h0 = 3 * j
        q0, k0, v0 = 64 * h0, D + 64 * h0, 2 * D + 64 * h0
        q01 = W_qkv[:, q0:q0 + 128]
        k01 = W_qkv[:, k0:k0 + 128] * KSCALE
        q2 = W_qkv[:, q0 + 128:q0 + 192]
        k2 = W_qkv[:, k0 + 128:k0 + 192] * KSCALE
        vh0 = W_qkv[:, v0:v0 + 64]
        vh1 = W_qkv[:, v0 + 64:v0 + 128]
        vh2 = W_qkv[:, v0 + 128:v0 + 192]
        wqkv_full = np.concatenate(
            [q01, k01, q2, k2, vh0, vh2, vh1], axis=1
        ).astype(np.float16)                      # [768, 576]
        wqkv_packed = np.ascontiguousarray(
            wqkv_full.reshape(DC, 128, WCOLS).transpose(1, 0, 2)
        ).reshape(128, DC * WCOLS)
        r0 = 64 * h0
        wout_slice = np.ascontiguousarray(
            W_out[r0:r0 + 192].astype(np.float16))
        xb = x[b].astype(np.float16)              # [N, 768]
        x_packed = np.ascontiguousarray(
            xb.reshape(NSEG, SEG, DC, 128).transpose(3, 0, 2, 1)
        ).reshape(128, NSEG, DC * SEG)
        in_maps.append({
            "x": x_packed,
            "wqkv": wqkv_packed,
            "wout": wout_slice,
        })

    nc = _get_nc()
    kwargs = {}
    if TRACE:
        from concourse import bass_utils as _bu
        _bu.upload_artifacts = lambda tmpdir: "local://" + tmpdir
        kwargs["trace"] = True
        if TRACE_ALL_CORES:
            kwargs["trace_cores"] = list(range(N_CORES))
    res = run_bass_kernel_spmd(nc, in_maps, core_ids=list(range(N_CORES)), **kwargs)
    LAST_RESULT = res

    out = np.empty((B, N, D), dtype=np.float32)
    for b in range(B):
        acc = (res.results[4 * b + 0]["y"].astype(np.float32)
               + res.results[4 * b + 1]["y"].astype(np.float32)
               + res.results[4 * b + 2]["y"].astype(np.float32)
               + res.results[4 * b + 3]["y"].astype(np.float32))
        out[b] = acc + b_out[None, :]
    return out


# revision 11
# speedup vs baseline: 1.0259x; 1.0259x over previous
"""Trainium2 Bass kernel v6 — paired score tiles + fp16 I/O + lean normalize.

Multi-head attention (B=2, N=4096, D=768, H=12, d_head=64) on 8 NeuronCores.
Data-parallel over batch (4 cores per element), tensor-parallel over heads
(3 heads per core). Host sums the 4 partial outputs per batch element and
adds the bias.

v6 changes over v4 (504628 ns):

1. fp16 host-side inputs: x, wqkv, wout are cast to fp16 on the host, so
   the input DMA halves and all on-chip fp32->fp16 casts disappear.
   y partials are DMA'd out as fp16 (summed in fp32 on host).

2. Score PSUM tiles hold ONE key chunk x BOTH heads (h0 in PE rows 0:64,
   h1 in rows 64:128). v4 grouped (one head x two chunks), so a PSUM-slot
   release enabled two same-half matmuls which serialized; now every slot
   release enables an (h0,h64) row-tiled pair that runs concurrently.
   The v4 trace showed ~100us lost to this serialization.

3. exp writes per-head planes via strided APs into P[parity, head, q]
   fp16 tiles; PV consumes per-head slices P[:, par, h, :].

4. Normalize chain: reciprocal_approx_fast reads the denominator row
   directly from PSUM (the v4 den-copy is dropped); ysb copies alternate
   Scalar/Vector.

(fp8 DoubleRow PV was tried and rejected: quantization noise on V/p does
not average down — attention outputs are themselves weighted means — so
end-to-end error lands at ~3.3e-2 vs the 2e-2 gate.)

Layouts (per core, fp16):
  kT01/qT01 [128, N]: h0 dims rows 0:64, h1 rows 64:128 (K pre-scaled by
      alpha*SCALE host-side).
  kT2/qT2   [128, N]: h2 duplicated into both partition halves.
  v16[h] [128, NKC, 128]: keys on partitions; h0/h2: cols 0:64 = V,
      col 64 = ones (softmax denominator); h1: col 0 = ones, cols 64:128 = V
      (so o1's data rows land at partitions 64:128, partition-aligned with
      A01's h1 half).
  P [128, 2, 2, QB] per chunk-pair: [keys, parity, head, q].
  A01 [128, N] (h0 rows 0:64, h1 rows 64:128), A2 [64, N].
wqkv host layout [768, 704]: [q01 | k01*KS | q2 q2 | k2*KS k2*KS | v012].
wout host layout [192, 768]: [W_h0;W_h1;W_h2]. Bias added on host.
"""

import numpy as np

import concourse.bass as bass
import concourse.tile as tile
from concourse import mybir, bacc
from concourse.bass_utils import run_bass_kernel_spmd

F32 = mybir.dt.float32
F16 = mybir.dt.float16
EXP = mybir.ActivationFunctionType.Exp

N_CORES = 8
B = 2
N = 4096
D = 768
H = 12
HD = 64
SCALE = HD ** -0.5
DC = D // 128        # 6 contraction chunks
NKC = N // 128       # 32 key chunks
NCP = NKC // 2       # 16 chunk pairs
QB = 512             # query block
NQB = N // QB        # 8

# EXP32 fit: g(u) = ((u + EXP_A)^2 + EXP_B)^32 ~= exp(u/ALPHA)
ALPHA = 0.0230805526
EXP_A = 0.6770127392
EXP_B = 0.5415557589
KSCALE = ALPHA * SCALE          # folded into K projection columns host-side
ACT_SCALE = 1.0 / ALPHA         # ScalarE: exp(ACT_SCALE * s)

DVE_SHARE = 0.45                # fraction of exp tiles routed to the DVE

TRACE = False
TRACE_ALL_CORES = False
LAST_RESULT = None

_nc_cache = None
_exp32_op = None


def _register_exp32():
    """Register the EXP32 custom DVE op at runtime (idempotent)."""
    global _exp32_op
    if _exp32_op is not None:
        return _exp32_op
    import concourse.dve_ops as dve_ops
    from concourse.dve_spec import Spec, Src0, C1, C2, sq, lower
    from concourse.dve_uop import DveOpSpec

    name = "EXP32_SQCHAIN"
    for op in dve_ops.OPS:
        if op.name == name:
            _exp32_op = op
            return op
    body = sq(sq(sq(sq(sq(sq(Src0 + C1) + C2)))))
    spec = Spec(
        body=body,
        reference=lambda in0, in1, s0, s1, imm2: (
            ((in0.astype(np.float32) + s1) ** 2 + imm2) ** 32
        ).astype(np.float32),
    )
    row = dve_ops._CUSTOM_DVE_ROW_BASE + len(dve_ops.OPS)
    assert row < 0x20
    shas = {
        ver: DveOpSpec(
            name=name, opcode=row, uops=lower(spec, ver=ver), rd1_en=False
        ).sha(ver)
        for ver in ("v3", "v4")
    }
    op = dve_ops.DveOp(name, spec, subdim=False, uops_sha=shas)
    dve_ops.OPS.append(op)
    dve_ops._SUB_OPCODE_FOR_NAME[name] = row
    dve_ops.CUSTOM_DVE_SPECS[name] = spec
    _exp32_op = op
    return op


def _build_module():
    nc = bacc.Bacc("TRN2", target_bir_lowering=False, debug=False,
                   num_devices=N_CORES)
    x_d = nc.dram_tensor("x", [D, N], F16, kind="ExternalInput")
    wqkv_d = nc.dram_tensor("wqkv", [D, 704], F16, kind="ExternalInput")
    wout_d = nc.dram_tensor("wout", [192, D], F16, kind="ExternalInput")
    y_d = nc.dram_tensor("y", [N, D], F16, kind="ExternalOutput")

    with tile.TileContext(nc) as tc:
        _emit(nc, tc, x_d, wqkv_d, wout_d, y_d)
    nc.compile()
    return nc


def _emit(nc, tc, x_d, wqkv_d, wout_d, y_d):
    from contextlib import ExitStack
    exp32 = _register_exp32()
    ctx = ExitStack()
    with ctx:
        weights = ctx.enter_context(tc.tile_pool(name="weights", bufs=1))
        qkvp = ctx.enter_context(tc.tile_pool(name="qkv", bufs=1))
        apool = ctx.enter_context(tc.tile_pool(name="attnout", bufs=1))

        # --- weights / persistent activations ---------------------------
        wqkv = weights.tile([128, DC, 704], F16, tag="wqkv")
        W01 = weights.tile([128, D], F16, tag="W01")
        W2 = weights.tile([64, D], F16, tag="W2")
        qT01 = qkvp.tile([128, N], F16, tag="qT01")
        kT01 = qkvp.tile([128, N], F16, tag="kT01")
        qT2 = qkvp.tile([128, N], F16, tag="qT2")
        kT2 = qkvp.tile([128, N], F16, tag="kT2")
        v16 = [qkvp.tile([128, NKC, 128], F16, tag=f"v{h}", name=f"v{h}")
               for h in range(3)]
        A01 = apool.tile([128, N], F16, tag="A01")
        A2 = apool.tile([64, N], F16, tag="A2")

        # one-time fills (GpSimd; overlapped with initial DMA).
        nc.gpsimd.memset(v16[0][:, :, 65:128], 0.0)
        nc.gpsimd.memset(v16[0][:, :, 64:65], 1.0)
        nc.gpsimd.memset(v16[1][:, :, 1:64], 0.0)
        nc.gpsimd.memset(v16[1][:, :, 0:1], 1.0)
        nc.gpsimd.memset(v16[2][:, :, 65:128], 0.0)
        nc.gpsimd.memset(v16[2][:, :, 64:65], 1.0)

        # ================= phase A: DMA + projections ====================
        with tc.tile_pool(name="xT", bufs=2) as xTp, \
             tc.tile_pool(name="vps", bufs=2, space=bass.MemorySpace.PSUM) as vps, \
             tc.tile_pool(name="qkps", bufs=2, space=bass.MemorySpace.PSUM) as qkps:
            nc.sync.dma_start(
                wqkv[:], wqkv_d.ap().rearrange("(c p) m -> p c m", p=128))
            nc.sync.dma_start(W01[:], wout_d.ap()[0:128, :])
            nc.sync.dma_start(W2[:], wout_d.ap()[128:192, :])

            NSEG = 4
            SEG = N // NSEG
            SEGC = SEG // 128
            qk_eng = [nc.vector, nc.scalar]
            qk_i = 0
            for seg in range(NSEG):
                t0 = seg * SEGC
                col0 = seg * SEG
                xT = xTp.tile([128, DC, SEG], F16, tag="xT")
                nc.sync.dma_start(
                    xT[:],
                    x_d.ap().rearrange("(c p) n -> p c n", p=128)
                    [:, :, col0:col0 + SEG])
                # k first so attention can start before q finishes
                for ci, dst in ((1, kT01), (3, kT2), (-1, None),
                                (0, qT01), (2, qT2)):
                    if ci == -1:
                        for t in range(SEGC):
                            kc = t0 + t
                            acc = vps.tile([128, 192], F32, tag="vps")
                            for c in range(DC):
                                nc.tensor.matmul(acc[:],
                                                 xT[:, c, t * 128:(t + 1) * 128],
                                                 wqkv[:, c, 512:704],
                                                 start=(c == 0), stop=(c == DC - 1))
                            nc.scalar.copy(v16[0][:, kc, 0:64], acc[:, 0:64])
                            nc.scalar.copy(v16[1][:, kc, 64:128], acc[:, 64:128])
                            nc.scalar.copy(v16[2][:, kc, 0:64], acc[:, 128:192])
                        continue
                    c0 = 128 * ci
                    for nb in range(SEG // 512):
                        acc = qkps.tile([128, 512], F32, tag="qkps")
                        for c in range(DC):
                            nc.tensor.matmul(acc[:], wqkv[:, c, c0:c0 + 128],
                                             xT[:, c, nb * 512:(nb + 1) * 512],
                                             start=(c == 0), stop=(c == DC - 1))
                        cc = col0 + nb * 512
                        eng = qk_eng[qk_i % 2]
                        qk_i += 1
                        if eng is nc.scalar:
                            nc.scalar.copy(dst[:, cc:cc + 512], acc[:])
                        else:
                            eng.tensor_copy(dst[:, cc:cc + 512], acc[:])

        # ========= phase B: attention + fused output projection ==========
        # PSUM budget (8 banks): tag "s" ring 3 x [128,2,QB] f32 (6 banks,
        # also hosts the [128,D] y-projection tiles) + tag "o" ring 2 x
        # [128,QB] f32 (2 banks).
        with tc.tile_pool(name="sps", bufs=3, space=bass.MemorySpace.PSUM) as sps, \
             tc.tile_pool(name="ops", bufs=2, space=bass.MemorySpace.PSUM) as ops, \
             tc.tile_pool(name="pp", bufs=3) as pp, \
             tc.tile_pool(name="p2p", bufs=4) as p2p, \
             tc.tile_pool(name="ysbp", bufs=3) as ysbp, \
             tc.tile_pool(name="rp", bufs=4) as rp, \
             tc.tile_pool(name="rbp", bufs=2) as rbp:
            pending = []       # deferred output-projection token chunks
            exp_ctr = [0]

            def emit_exp(p_ap, s_ap, force_act=False):
                # force_act: exps near a loop end go to ScalarE so the DVE
                # queue is empty when the normalize chain needs it.
                if force_act:
                    nc.scalar.activation(p_ap, s_ap, EXP, scale=ACT_SCALE)
                    return
                i = exp_ctr[0]
                exp_ctr[0] += 1
                if int((i + 1) * DVE_SHARE) - int(i * DVE_SHARE) == 1:
                    nc.vector._custom_dve(exp32, out=p_ap, in0=s_ap,
                                          s1=EXP_A, imm2=EXP_B)
                else:
                    nc.scalar.activation(p_ap, s_ap, EXP, scale=ACT_SCALE)

            y_eng = [0]

            def emit_y(t):
                ts = slice(t * 128, (t + 1) * 128)
                y = sps.tile([128, D], F32, tag="s", name="y")
                for c0, c1 in ((0, 512), (512, 768)):
                    nc.tensor.matmul(y[:, c0:c1], A01[:, ts], W01[:, c0:c1],
                                     start=True, stop=False)
                    nc.tensor.matmul(y[:, c0:c1], A2[:, ts], W2[:, c0:c1],
                                     start=False, stop=True)
                ysb = ysbp.tile([128, D], F16, tag="ysb", name="ysb")
                if y_eng[0] % 2 == 0:
                    nc.scalar.copy(ysb[:], y[:])
                else:
                    nc.vector.tensor_copy(ysb[:], y[:])
                y_eng[0] += 1
                nc.sync.dma_start(y_d.ap()[ts, :], ysb[:])

            def normalize(o, den_row, dst, o_rows, bcast_rows):
                # custom-DVE recip must read base-partition-0 SBUF, so copy
                # the PSUM denominator row out first (reading PSUM@base64
                # directly yields garbage -> unnormalized output).
                den = rp.tile([1, QB], F32, tag="den", name="den")
                nc.vector.tensor_copy(den[:], o[den_row:den_row + 1, :])
                rc = rp.tile([1, QB], F32, tag="rc", name="rc")
                nc.vector.reciprocal_approx_fast(rc[:], den[:])
                # partition_broadcast only writes correctly into base-0 APs,
                # so broadcast into a full 128-row tile and slice on consume.
                rcb = rbp.tile([128, QB], F32, tag="rcb", name="rcb")
                nc.gpsimd.partition_broadcast(rcb[:], rc[:])
                nc.vector.tensor_mul(dst, o[o_rows, :], rcb[bcast_rows, :])

            for qb in range(NQB):
                q0 = qb * QB
                qs = slice(q0, q0 + QB)
                # ---- heads 0+1: paired score tiles, PV lagged one pair --
                o0 = ops.tile([128, QB], F32, tag="o", name="o0")
                o1 = ops.tile([128, QB], F32, tag="o", name="o1")
                prev = None
                for cp in range(NCP):
                    P = pp.tile([128, 2, 2, QB], F16, tag="p", name="P")
                    tail = cp >= NCP - 2
                    for par in (0, 1):
                        kc = 2 * cp + par
                        ks = slice(kc * 128, (kc + 1) * 128)
                        s = sps.tile([128, 2, QB], F32, tag="s", name="s")
                        nc.tensor.matmul(s[:, 0, :], kT01[0:64, ks],
                                         qT01[0:64, qs], start=True, stop=True)
                        nc.tensor.matmul(s[:, 1, :], kT01[64:128, ks],
                                         qT01[64:128, qs], start=True, stop=True)
                        emit_exp(P[:, par, :, :], s[:], force_act=tail)
                    if prev is not None:
                        pP, pcp = prev
                        for par in (0, 1):
                            st = (pcp == 0 and par == 0)
                            nc.tensor.matmul(o0[:], v16[0][:, 2 * pcp + par, :],
                                             pP[:, par, 0, :], start=st,
                                             stop=False)
                            nc.tensor.matmul(o1[:], v16[1][:, 2 * pcp + par, :],
                                             pP[:, par, 1, :], start=st,
                                             stop=False)
                    prev = (P, cp)
                pP, pcp = prev
                for par in (0, 1):
                    sp = par == 1
                    nc.tensor.matmul(o0[:], v16[0][:, 2 * pcp + par, :],
                                     pP[:, par, 0, :], start=False, stop=sp)
                    nc.tensor.matmul(o1[:], v16[1][:, 2 * pcp + par, :],
                                     pP[:, par, 1, :], start=False, stop=sp)
                # normalize h0 -> A01 rows 0:64, h1 -> rows 64:128
                normalize(o0, 64, A01[0:64, qs], slice(0, 64), slice(0, 64))
                normalize(o1, 0, A01[64:128, qs], slice(64, 128),
                          slice(64, 128))
                # ---- head 2: dual-chunk row tiling, PV lagged depth 2 ----
                o2 = ops.tile([128, QB], F32, tag="o", name="o2")
                lag2 = []
                for cp in range(NCP):
                    s2 = sps.tile([128, 2, QB], F32, tag="s", name="s2")
                    ka = slice((2 * cp) * 128, (2 * cp + 1) * 128)
                    kb = slice((2 * cp + 1) * 128, (2 * cp + 2) * 128)
                    nc.tensor.matmul(s2[:, 0, :], kT2[0:64, ka],
                                     qT2[0:64, qs], start=True, stop=True)
                    nc.tensor.matmul(s2[:, 1, :], kT2[64:128, kb],
                                     qT2[64:128, qs], start=True, stop=True)
                    p2 = p2p.tile([128, 2, QB], F16, tag="p2", name="p2")
                    emit_exp(p2[:], s2[:], force_act=(cp >= NCP - 2))
                    lag2.append((p2, cp))
                    if len(lag2) > 2:
                        pp2, pc = lag2.pop(0)
                        for par in (0, 1):
                            nc.tensor.matmul(o2[:], v16[2][:, 2 * pc + par, :],
                                             pp2[:, par, :],
                                             start=(pc == 0 and par == 0),
                                             stop=False)
                    if pending and cp in (2, 6, 10, 14):
                        emit_y(pending.pop(0))
                for pp2, pc in lag2:
                    for par in (0, 1):
                        nc.tensor.matmul(o2[:], v16[2][:, 2 * pc + par, :],
                                         pp2[:, par, :], start=False,
                                         stop=(pc == NCP - 1 and par == 1))
                normalize(o2, 64, A2[0:64, qs], slice(0, 64), slice(0, 64))
                pending.extend(range(qb * (QB // 128), (qb + 1) * (QB // 128)))
            for t in pending:
                emit_y(t)


def _get_nc():
    global _nc_cache
    if _nc_cache is None:
        _nc_cache = _build_module()
    return _nc_cache


def kernel(x, W_qkv, W_out, b_out):
    global LAST_RESULT
    x = np.asarray(x, dtype=np.float32)
    W_qkv = np.asarray(W_qkv, dtype=np.float32)
    W_out = np.asarray(W_out, dtype=np.float32)
    b_out = np.asarray(b_out, dtype=np.float32)

    in_maps = []
    for c in range(N_CORES):
        b, j = divmod(c, 4)
        h0 = 3 * j
        q0, k0, v0 = 64 * h0, D + 64 * h0, 2 * D + 64 * h0
        q01 = W_qkv[:, q0:q0 + 128]
        k01 = W_qkv[:, k0:k0 + 128] * KSCALE
        q2 = W_qkv[:, q0 + 128:q0 + 192]
        k2 = W_qkv[:, k0 + 128:k0 + 192] * KSCALE
        v012 = W_qkv[:, v0:v0 + 192]
        wqkv_slice = np.ascontiguousarray(
            np.concatenate([q01, k01, q2, q2, k2, k2, v012],
                           axis=1).astype(np.float16))
        r0 = 64 * h0
        wout_slice = np.ascontiguousarray(
            W_out[r0:r0 + 192].astype(np.float16))
        in_maps.append({
            "x": np.ascontiguousarray(x[b].T.astype(np.float16)),
            "wqkv": wqkv_slice,
            "wout": wout_slice,
        })

    nc = _get_nc()
    kwargs = {}
    if TRACE:
        from concourse import bass_utils as _bu
        _bu.upload_artifacts = lambda tmpdir: "local://" + tmpdir
        kwargs["trace"] = True
        if TRACE_ALL_CORES:
            kwargs["trace_cores"] = list(range(N_CORES))
    res = run_bass_kernel_spmd(nc, in_maps, core_ids=list(range(N_CORES)), **kwargs)
    LAST_RESULT = res

    out = np.empty((B, N, D), dtype=np.float32)
    for b in range(B):
        acc = (res.results[4 * b + 0]["y"].astype(np.float32)
               + res.results[4 * b + 1]["y"].astype(np.float32)
               + res.results[4 * b + 2]["y"].astype(np.float32)
               + res.results[4 * b + 3]["y"].astype(np.float32))
        out[b] = acc + b_out[None, :]
    return out


# revision 13
# speedup vs baseline: 1.0273x; 1.0014x over previous
"""Trainium2 Bass kernel v6 — paired score tiles + fp16 I/O + lean normalize.

Multi-head attention (B=2, N=4096, D=768, H=12, d_head=64) on 8 NeuronCores.
Data-parallel over batch (4 cores per element), tensor-parallel over heads
(3 heads per core). Host sums the 4 partial outputs per batch element and
adds the bias.

v6 changes over v4 (504628 ns):

1. fp16 host-side inputs: x, wqkv, wout are cast to fp16 on the host, so
   the input DMA halves and all on-chip fp32->fp16 casts disappear.
   y partials are DMA'd out as fp16 (summed in fp32 on host).

2. Score PSUM tiles hold ONE key chunk x BOTH heads (h0 in PE rows 0:64,
   h1 in rows 64:128). v4 grouped (one head x two chunks), so a PSUM-slot
   release enabled two same-half matmuls which serialized; now every slot
   release enables an (h0,h64) row-tiled pair that runs concurrently.
   The v4 trace showed ~100us lost to this serialization.

3. exp writes per-head planes via strided APs into P[parity, head, q]
   fp16 tiles; PV consumes per-head slices P[:, par, h, :].

4. Normalize chain: reciprocal_approx_fast reads the denominator row
   directly from PSUM (the v4 den-copy is dropped); ysb copies alternate
   Scalar/Vector.

(fp8 DoubleRow PV was tried and rejected: quantization noise on V/p does
not average down — attention outputs are themselves weighted means — so
end-to-end error lands at ~3.3e-2 vs the 2e-2 gate.)

Layouts (per core, fp16):
  kT01/qT01 [128, N]: h0 dims rows 0:64, h1 rows 64:128 (K pre-scaled by
      alpha*SCALE host-side).
  kT2/qT2   [128, N]: h2 duplicated into both partition halves.
  v16[h] [128, NKC, 128]: keys on partitions; h0/h2: cols 0:64 = V,
      col 64 = ones (softmax denominator); h1: col 0 = ones, cols 64:128 = V
      (so o1's data rows land at partitions 64:128, partition-aligned with
      A01's h1 half).
  P [128, 2, 2, QB] per chunk-pair: [keys, parity, head, q].
  A01 [128, N] (h0 rows 0:64, h1 rows 64:128), A2 [64, N].
wqkv host layout [768, 704]: [q01 | k01*KS | q2 q2 | k2*KS k2*KS | v012].
wout host layout [192, 768]: [W_h0;W_h1;W_h2]. Bias added on host.
"""

import numpy as np

import concourse.bass as bass
import concourse.tile as tile
from concourse import mybir, bacc
from concourse.bass_utils import run_bass_kernel_spmd

F32 = mybir.dt.float32
F16 = mybir.dt.float16
EXP = mybir.ActivationFunctionType.Exp

N_CORES = 8
B = 2
N = 4096
D = 768
H = 12
HD = 64
SCALE = HD ** -0.5
DC = D // 128        # 6 contraction chunks
NKC = N // 128       # 32 key chunks
NCP = NKC // 2       # 16 chunk pairs
QB = 512             # query block
NQB = N // QB        # 8

# EXP32 fit: g(u) = ((u + EXP_A)^2 + EXP_B)^32 ~= exp(u/ALPHA)
ALPHA = 0.0230805526
EXP_A = 0.6770127392
EXP_B = 0.5415557589
KSCALE = ALPHA * SCALE          # folded into K projection columns host-side
ACT_SCALE = 1.0 / ALPHA         # ScalarE: exp(ACT_SCALE * s)

DVE_SHARE = 0.48                # fraction of exp tiles routed to the DVE
                                # (v6.1 trace: Scalar 303us busy vs DVE 258)

TRACE = False
TRACE_ALL_CORES = False
LAST_RESULT = None

_nc_cache = None
_exp32_op = None


def _register_exp32():
    """Register the EXP32 custom DVE op at runtime (idempotent)."""
    global _exp32_op
    if _exp32_op is not None:
        return _exp32_op
    import concourse.dve_ops as dve_ops
    from concourse.dve_spec import Spec, Src0, C1, C2, sq, lower
    from concourse.dve_uop import DveOpSpec

    name = "EXP32_SQCHAIN"
    for op in dve_ops.OPS:
        if op.name == name:
            _exp32_op = op
            return op
    body = sq(sq(sq(sq(sq(sq(Src0 + C1) + C2)))))
    spec = Spec(
        body=body,
        reference=lambda in0, in1, s0, s1, imm2: (
            ((in0.astype(np.float32) + s1) ** 2 + imm2) ** 32
        ).astype(np.float32),
    )
    row = dve_ops._CUSTOM_DVE_ROW_BASE + len(dve_ops.OPS)
    assert row < 0x20
    shas = {
        ver: DveOpSpec(
            name=name, opcode=row, uops=lower(spec, ver=ver), rd1_en=False
        ).sha(ver)
        for ver in ("v3", "v4")
    }
    op = dve_ops.DveOp(name, spec, subdim=False, uops_sha=shas)
    dve_ops.OPS.append(op)
    dve_ops._SUB_OPCODE_FOR_NAME[name] = row
    dve_ops.CUSTOM_DVE_SPECS[name] = spec
    _exp32_op = op
    return op


def _build_module():
    nc = bacc.Bacc("TRN2", target_bir_lowering=False, debug=False,
                   num_devices=N_CORES)
    x_d = nc.dram_tensor("x", [D, N], F16, kind="ExternalInput")
    wqkv_d = nc.dram_tensor("wqkv", [D, 704], F16, kind="ExternalInput")
    wout_d = nc.dram_tensor("wout", [192, D], F16, kind="ExternalInput")
    y_d = nc.dram_tensor("y", [N, D], F16, kind="ExternalOutput")

    with tile.TileContext(nc) as tc:
        _emit(nc, tc, x_d, wqkv_d, wout_d, y_d)
    nc.compile()
    return nc


def _emit(nc, tc, x_d, wqkv_d, wout_d, y_d):
    from contextlib import ExitStack
    exp32 = _register_exp32()
    ctx = ExitStack()
    with ctx:
        weights = ctx.enter_context(tc.tile_pool(name="weights", bufs=1))
        qkvp = ctx.enter_context(tc.tile_pool(name="qkv", bufs=1))
        apool = ctx.enter_context(tc.tile_pool(name="attnout", bufs=1))

        # --- weights / persistent activations ---------------------------
        wqkv = weights.tile([128, DC, 704], F16, tag="wqkv")
        W01 = weights.tile([128, D], F16, tag="W01")
        W2 = weights.tile([64, D], F16, tag="W2")
        qT01 = qkvp.tile([128, N], F16, tag="qT01")
        kT01 = qkvp.tile([128, N], F16, tag="kT01")
        qT2 = qkvp.tile([128, N], F16, tag="qT2")
        kT2 = qkvp.tile([128, N], F16, tag="kT2")
        v16 = [qkvp.tile([128, NKC, 128], F16, tag=f"v{h}", name=f"v{h}")
               for h in range(3)]
        A01 = apool.tile([128, N], F16, tag="A01")
        A2 = apool.tile([64, N], F16, tag="A2")

        # one-time fills (GpSimd; overlapped with initial DMA).
        nc.gpsimd.memset(v16[0][:, :, 65:128], 0.0)
        nc.gpsimd.memset(v16[0][:, :, 64:65], 1.0)
        nc.gpsimd.memset(v16[1][:, :, 1:64], 0.0)
        nc.gpsimd.memset(v16[1][:, :, 0:1], 1.0)
        nc.gpsimd.memset(v16[2][:, :, 65:128], 0.0)
        nc.gpsimd.memset(v16[2][:, :, 64:65], 1.0)

        # ================= phase A: DMA + projections ====================
        with tc.tile_pool(name="xT", bufs=2) as xTp, \
             tc.tile_pool(name="vps", bufs=2, space=bass.MemorySpace.PSUM) as vps, \
             tc.tile_pool(name="qkps", bufs=2, space=bass.MemorySpace.PSUM) as qkps:
            nc.sync.dma_start(
                wqkv[:], wqkv_d.ap().rearrange("(c p) m -> p c m", p=128))
            nc.sync.dma_start(W01[:], wout_d.ap()[0:128, :])
            nc.sync.dma_start(W2[:], wout_d.ap()[128:192, :])

            NSEG = 4
            SEG = N // NSEG
            SEGC = SEG // 128
            qk_eng = [nc.vector, nc.scalar]
            qk_i = 0
            for seg in range(NSEG):
                t0 = seg * SEGC
                col0 = seg * SEG
                xT = xTp.tile([128, DC, SEG], F16, tag="xT")
                nc.sync.dma_start(
                    xT[:],
                    x_d.ap().rearrange("(c p) n -> p c n", p=128)
                    [:, :, col0:col0 + SEG])
                # k first so attention can start before q finishes
                for ci, dst in ((1, kT01), (3, kT2), (-1, None),
                                (0, qT01), (2, qT2)):
                    if ci == -1:
                        for t in range(SEGC):
                            kc = t0 + t
                            acc = vps.tile([128, 192], F32, tag="vps")
                            for c in range(DC):
                                nc.tensor.matmul(acc[:],
                                                 xT[:, c, t * 128:(t + 1) * 128],
                                                 wqkv[:, c, 512:704],
                                                 start=(c == 0), stop=(c == DC - 1))
                            nc.scalar.copy(v16[0][:, kc, 0:64], acc[:, 0:64])
                            nc.vector.tensor_copy(v16[1][:, kc, 64:128],
                                                  acc[:, 64:128])
                            nc.scalar.copy(v16[2][:, kc, 0:64], acc[:, 128:192])
                        continue
                    c0 = 128 * ci
                    for nb in range(SEG // 512):
                        acc = qkps.tile([128, 512], F32, tag="qkps")
                        for c in range(DC):
                            nc.tensor.matmul(acc[:], wqkv[:, c, c0:c0 + 128],
                                             xT[:, c, nb * 512:(nb + 1) * 512],
                                             start=(c == 0), stop=(c == DC - 1))
                        cc = col0 + nb * 512
                        eng = qk_eng[qk_i % 2]
                        qk_i += 1
                        if eng is nc.scalar:
                            nc.scalar.copy(dst[:, cc:cc + 512], acc[:])
                        else:
                            eng.tensor_copy(dst[:, cc:cc + 512], acc[:])

        # ========= phase B: attention + fused output projection ==========
        # PSUM budget (8 banks): tag "s" ring 3 x [128,2,QB] f32 (6 banks,
        # also hosts the [128,D] y-projection tiles) + tag "o" ring 2 x
        # [128,QB] f32 (2 banks).
        with tc.tile_pool(name="sps", bufs=3, space=bass.MemorySpace.PSUM) as sps, \
             tc.tile_pool(name="ops", bufs=2, space=bass.MemorySpace.PSUM) as ops, \
             tc.tile_pool(name="pp", bufs=3) as pp, \
             tc.tile_pool(name="p2p", bufs=4) as p2p, \
             tc.tile_pool(name="ysbp", bufs=3) as ysbp, \
             tc.tile_pool(name="rp", bufs=4) as rp, \
             tc.tile_pool(name="rbp", bufs=2) as rbp:
            pending = []       # deferred output-projection token chunks
            exp_ctr = [0]

            def emit_exp(p_ap, s_ap, force_act=False):
                # force_act: exps near a loop end go to ScalarE so the DVE
                # queue is empty when the normalize chain needs it.
                if force_act:
                    nc.scalar.activation(p_ap, s_ap, EXP, scale=ACT_SCALE)
                    return
                i = exp_ctr[0]
                exp_ctr[0] += 1
                if int((i + 1) * DVE_SHARE) - int(i * DVE_SHARE) == 1:
                    nc.vector._custom_dve(exp32, out=p_ap, in0=s_ap,
                                          s1=EXP_A, imm2=EXP_B)
                else:
                    nc.scalar.activation(p_ap, s_ap, EXP, scale=ACT_SCALE)

            y_eng = [0]

            def emit_y(t):
                ts = slice(t * 128, (t + 1) * 128)
                y = sps.tile([128, D], F32, tag="s", name="y")
                for c0, c1 in ((0, 512), (512, 768)):
                    nc.tensor.matmul(y[:, c0:c1], A01[:, ts], W01[:, c0:c1],
                                     start=True, stop=False)
                    nc.tensor.matmul(y[:, c0:c1], A2[:, ts], W2[:, c0:c1],
                                     start=False, stop=True)
                ysb = ysbp.tile([128, D], F16, tag="ysb", name="ysb")
                if y_eng[0] % 2 == 0:
                    nc.scalar.copy(ysb[:], y[:])
                else:
                    nc.vector.tensor_copy(ysb[:], y[:])
                y_eng[0] += 1
                nc.sync.dma_start(y_d.ap()[ts, :], ysb[:])

            def normalize(o, den_row, dst, o_rows, bcast_rows):
                # custom-DVE recip must read base-partition-0 SBUF, so copy
                # the PSUM denominator row out first (reading PSUM@base64
                # directly yields garbage -> unnormalized output).
                den = rp.tile([1, QB], F32, tag="den", name="den")
                nc.vector.tensor_copy(den[:], o[den_row:den_row + 1, :])
                rc = rp.tile([1, QB], F32, tag="rc", name="rc")
                nc.vector.reciprocal_approx_fast(rc[:], den[:])
                # partition_broadcast only writes correctly into base-0 APs,
                # so broadcast into a full 128-row tile and slice on consume.
                rcb = rbp.tile([128, QB], F32, tag="rcb", name="rcb")
                nc.gpsimd.partition_broadcast(rcb[:], rc[:])
                nc.vector.tensor_mul(dst, o[o_rows, :], rcb[bcast_rows, :])

            for qb in range(NQB):
                q0 = qb * QB
                qs = slice(q0, q0 + QB)
                # ---- heads 0+1: paired score tiles, PV lagged one pair --
                o0 = ops.tile([128, QB], F32, tag="o", name="o0")
                o1 = ops.tile([128, QB], F32, tag="o", name="o1")
                prev = None
                for cp in range(NCP):
                    P = pp.tile([128, 2, 2, QB], F16, tag="p", name="P")
                    tail = cp >= NCP - 2
                    for par in (0, 1):
                        kc = 2 * cp + par
                        ks = slice(kc * 128, (kc + 1) * 128)
                        s = sps.tile([128, 2, QB], F32, tag="s", name="s")
                        nc.tensor.matmul(s[:, 0, :], kT01[0:64, ks],
                                         qT01[0:64, qs], start=True, stop=True)
                        nc.tensor.matmul(s[:, 1, :], kT01[64:128, ks],
                                         qT01[64:128, qs], start=True, stop=True)
                        emit_exp(P[:, par, :, :], s[:], force_act=tail)
                    if prev is not None:
                        pP, pcp = prev
                        for par in (0, 1):
                            st = (pcp == 0 and par == 0)
                            nc.tensor.matmul(o0[:], v16[0][:, 2 * pcp + par, :],
                                             pP[:, par, 0, :], start=st,
                                             stop=False)
                            nc.tensor.matmul(o1[:], v16[1][:, 2 * pcp + par, :],
                                             pP[:, par, 1, :], start=st,
                                             stop=False)
                    prev = (P, cp)
                pP, pcp = prev
                for par in (0, 1):
                    sp = par == 1
                    nc.tensor.matmul(o0[:], v16[0][:, 2 * pcp + par, :],
                                     pP[:, par, 0, :], start=False, stop=sp)
                    nc.tensor.matmul(o1[:], v16[1][:, 2 * pcp + par, :],
                                     pP[:, par, 1, :], start=False, stop=sp)
                # normalize h0 -> A01 rows 0:64, h1 -> rows 64:128
                normalize(o0, 64, A01[0:64, qs], slice(0, 64), slice(0, 64))
                normalize(o1, 0, A01[64:128, qs], slice(64, 128),
                          slice(64, 128))
                # ---- head 2: dual-chunk row tiling, PV lagged depth 2 ----
                o2 = ops.tile([128, QB], F32, tag="o", name="o2")
                lag2 = []
                for cp in range(NCP):
                    s2 = sps.tile([128, 2, QB], F32, tag="s", name="s2")
                    ka = slice((2 * cp) * 128, (2 * cp + 1) * 128)
                    kb = slice((2 * cp + 1) * 128, (2 * cp + 2) * 128)
                    nc.tensor.matmul(s2[:, 0, :], kT2[0:64, ka],
                                     qT2[0:64, qs], start=True, stop=True)
                    nc.tensor.matmul(s2[:, 1, :], kT2[64:128, kb],
                                     qT2[64:128, qs], start=True, stop=True)
                    p2 = p2p.tile([128, 2, QB], F16, tag="p2", name="p2")
                    emit_exp(p2[:], s2[:], force_act=(cp >= NCP - 2))
                    lag2.append((p2, cp))
                    if len(lag2) > 2:
                        pp2, pc = lag2.pop(0)
                        for par in (0, 1):
                            nc.tensor.matmul(o2[:], v16[2][:, 2 * pc + par, :],
                                             pp2[:, par, :],
                                             start=(pc == 0 and par == 0),
                                             stop=False)
                    if pending and cp in (2, 6, 10, 14):
                        emit_y(pending.pop(0))
                for pp2, pc in lag2:
                    for par in (0, 1):
                        nc.tensor.matmul(o2[:], v16[2][:, 2 * pc + par, :],
                                         pp2[:, par, :], start=False,
                                         stop=(pc == NCP - 1 and par == 1))
                normalize(o2, 64, A2[0:64, qs], slice(0, 64), slice(0, 64))
                pending.extend(range(qb * (QB // 128), (qb + 1) * (QB // 128)))
            for t in pending:
                emit_y(t)


def _get_nc():
    global _nc_cache
    if _nc_cache is None:
        _nc_cache = _build_module()
    return _nc_cache


def kernel(x, W_qkv, W_out, b_out):
    global LAST_RESULT
    x = np.asarray(x, dtype=np.float32)
    W_qkv = np.asarray(W_qkv, dtype=np.float32)
    W_out = np.asarray(W_out, dtype=np.float32)
    b_out = np.asarray(b_out, dtype=np.float32)

    in_maps = []
    for c in range(N_CORES):
        b, j = divmod(c, 4)
        h0 = 3 * j
        q0, k0, v0 = 64 * h0, D + 64 * h0, 2 * D + 64 * h0
        q01 = W_qkv[:, q0:q0 + 128]
        k01 = W_qkv[:, k0:k0 + 128] * KSCALE
        q2 = W_qkv[:, q0 + 128:q0 + 192]
        k2 = W_qkv[:, k0 + 128:k0 + 192] * KSCALE
        v012 = W_qkv[:, v0:v0 + 192]
        wqkv_slice = np.ascontiguousarray(
            np.concatenate([q01, k01, q2, q2, k2, k2, v012],
                           axis=1).astype(np.float16))
        r0 = 64 * h0
        wout_slice = np.ascontiguousarray(
            W_out[r0:r0 + 192].astype(np.float16))
        in_maps.append({
            "x": np.ascontiguousarray(x[b].T.astype(np.float16)),
            "wqkv": wqkv_slice,
            "wout": wout_slice,
        })

    nc = _get_nc()
    kwargs = {}
    if TRACE:
        from concourse import bass_utils as _bu
        _bu.upload_artifacts = lambda tmpdir: "local://" + tmpdir
        kwargs["trace"] = True
        if TRACE_ALL_CORES:
            kwargs["trace_cores"] = list(range(N_CORES))
    res = run_bass_kernel_spmd(nc, in_maps, core_ids=list(range(N_CORES)), **kwargs)
    LAST_RESULT = res

    out = np.empty((B, N, D), dtype=np.float32)
    for b in range(B):
        acc = (res.results[4 * b + 0]["y"].astype(np.float32)
               + res.results[4 * b + 1]["y"].astype(np.float32)
               + res.results[4 * b + 2]["y"].astype(np.float32)
               + res.results[4 * b + 3]["y"].astype(np.float32))
        out[b] = acc + b_out[None, :]
    return out
